# revision 21
# baseline (speedup 1.0000x reference)
"""Trainium2 Bass kernel for nn_E4_C4 (C4-equivariant involution CNN).

Contract: kernel(**inputs) takes FULL unsharded inputs (as produced by
setup_inputs) and returns the FULL output [8, 512, 32, 32] fp32.

Strategy (data-parallel over batch, 1 batch element per core, 8 cores):
  per core, channels on partitions, spatial tap-shifts as free-dim offsets
  into a zero-padded v (40-wide rows, two x-parity copies so every tap's
  innermost reads are 4B-aligned for the DVE 2x bf16 mode):
    1. t  = W1 @ x           (PE GEMM, fp32r)
    2. GroupNorm+ReLU        (DVE bn_stats + tiny PE grouping matmuls +
                              ACT per-partition scale/bias apply) -> t1 bf16
    3. v  = Wv @ x           (PE GEMM) -> bf16 zero-padded 38x40 planes,
                              even-parity copy by ACT, odd-parity by DMA
    4. involution, r-pairs (0,1) and (2,3) interleaved so the K=64 wrep
       GEMMs of the two rotations sit in opposite PE row-groups (SBUF
       partitions 0-63 vs 64-127) and can row-tile concurrently on HW:
       per unit (r, tap p, spatial half nh):
         wrep = c2rep_p @ t1            (PE, bf16, N=512, K=64)
         route A: ACT evicts wrep+bias -> bf16 SBUF; DVE bf16 2x multiply
         route B: DVE STT (wrep+bias)*v from PSUM (1x)
         route C: Pool STT from PSUM (f32r product)
         out_ps += I.T @ prod           (PE identity-matmul accumulate)
       psW = single-bank [128,512] tiles, bufs=4 -> the wrep GEMMs run
       ~4 units ahead of the products; routes are interleaved to balance
       ACT/DVE/Pool occupancy.
  Host side: C4-lift of the 1x1 weights, channel reorders, replication of
  c2_w rows (fused into the wrep GEMM), rot90 tap permutation per r;
  final gather + channel re-order to reference layout.
"""

import math
import os
from contextlib import ExitStack

import numpy as np

import concourse.bacc as bacc
import concourse.bass as bass
import concourse.tile as tile
from concourse import mybir
from concourse.bass_utils import run_bass_kernel_spmd

# ---- problem constants (hardcoded per contract) ----
B = 8
CIN = 128
COUT = 128
KK = 7
R = 2
G = 8
GC = 16
H = W = 32
S = H * W  # 1024
EPS = 1e-5
NCORES = 8
VROW = 40  # padded v row stride (even => 4B-aligned bf16 rows)
VPLANE = 38 * VROW

PSW_BUFS = int(os.environ.get("KRN_PSW_BUFS", "3"))
PSO_BUFS = int(os.environ.get("KRN_PSO_BUFS", "1"))
PP_BUFS = int(os.environ.get("KRN_PP_BUFS", "8"))
POOL_UNITS = int(os.environ.get("KRN_POOL_UNITS", "10"))  # of 196
DVEB_UNITS = int(os.environ.get("KRN_DVEB_UNITS", "75"))  # route B units

F32 = mybir.dt.float32
F32R_G = mybir.dt.float32r
BF16 = mybir.dt.bfloat16


# ------------------------------------------------------------------ host prep
def _c4_lift_np(w):
    Wr = np.stack([np.roll(w, r, axis=-1) for r in range(4)], axis=1)  # [o,4,i,4]
    o, _, i, _ = Wr.shape
    return Wr.reshape(o * 4, i * 4)


def _host_prep(v_w, c1_w, gn_g, gn_b, c2_w, c2_b):
    W1 = _c4_lift_np(np.asarray(c1_w, np.float32))  # [256, 512], rows c*4+r
    # rows c*4+r -> r-major (r*64+c)
    W1_r = W1.reshape(64, 4, 512).transpose(1, 0, 2).reshape(256, 512)
    W1T = np.ascontiguousarray(W1_r.T)  # [512, 256]

    Wv = _c4_lift_np(np.asarray(v_w, np.float32))  # [512, 512], rows (g*16+c)*4+r
    Wv_r = Wv.reshape(128, 4, 512).transpose(1, 0, 2).reshape(512, 512)
    WvT = np.ascontiguousarray(Wv_r.T)  # [512, 512]

    gam_r = np.ascontiguousarray(
        np.asarray(gn_g, np.float32).reshape(64, 4).T.reshape(2, 128).T
    )  # [128, 2]  col t holds channels t*128..t*128+127 in r-major order
    bet_r = np.ascontiguousarray(
        np.asarray(gn_b, np.float32).reshape(64, 4).T.reshape(2, 128).T
    )

    c2_w = np.asarray(c2_w, np.float32)
    c2_b = np.asarray(c2_b, np.float32)
    c2rep = np.zeros((128, 2, 49, 128), np.float32)
    bias_rep = np.zeros((128, 4, 49), np.float32)
    m_idx = np.arange(128)
    for r in range(4):
        perm = np.rot90(np.arange(49).reshape(7, 7), k=r).flatten()
        base = 64 * (r % 2)
        slot = r // 2
        for p in range(49):
            src_rows = (m_idx // 16) * 49 + perm[p]
            c2rep[base : base + 64, slot, p, :] = c2_w[src_rows, :].T
            bias_rep[:, r, p] = c2_b[src_rows]

    i128 = np.eye(128, dtype=np.float32)
    gmat = np.zeros((128, 64), np.float32)
    gmat[np.arange(128), np.arange(128) % 64] = 0.25
    emat = np.zeros((64, 128), np.float32)
    emat[np.arange(128) % 64, np.arange(128)] = 1.0
    return W1T, WvT, gam_r, bet_r, c2rep, bias_rep, i128, gmat, emat


def _unit_routes():
    """Deterministic route per unit index (chains of 98 per r-pair stream).

    Units are (p, r) in emission order; returns list of 'A'|'B'|'C'."""
    total = 196
    n_c = min(POOL_UNITS, total)
    n_b = min(DVEB_UNITS, total - n_c)
    routes = []
    acc_c = 0.0
    acc_b = 0.0
    fc = n_c / total
    fb = n_b / (total - n_c) if total > n_c else 0.0
    for u in range(total):
        acc_c += fc
        if acc_c >= 1.0:
            acc_c -= 1.0
            routes.append("C")
            continue
        acc_b += fb
        if acc_b >= 1.0:
            acc_b -= 1.0
            routes.append("B")
        else:
            routes.append("A")
    # per 98-unit chain: tail drains on route B (shortest wrep->prod->id
    # chain), no Pool in the last TAIL slots nor the first 2
    TAIL = 6
    TAILB = 3
    for c0 in range(0, total, 98):
        chain = routes[c0 : c0 + 98]
        for i in list(range(98 - TAIL, 98)) + [0, 1]:
            if chain[i] == "C":
                for j in range(2, 98 - TAIL):
                    if chain[j] != "C":
                        chain[i], chain[j] = chain[j], chain[i]
                        break
        for i in range(98 - TAILB, 98):
            if chain[i] == "A":
                for j in range(2, 98 - TAILB):
                    if chain[j] == "B":
                        chain[i], chain[j] = chain[j], chain[i]
                        break
        routes[c0 : c0 + 98] = chain
    return routes


# ------------------------------------------------------------------ bass build
def _build_module(loop_n=1):
    nc = bacc.Bacc(None)

    x_d = nc.dram_tensor("x", [512, S], BF16, kind="ExternalInput")
    w1t_d = nc.dram_tensor("w1t", [512, 256], BF16, kind="ExternalInput")
    wvt_d = nc.dram_tensor("wvt", [512, 512], BF16, kind="ExternalInput")
    c2r_d = nc.dram_tensor("c2rep", [128, 2, 49, 128], BF16, kind="ExternalInput")
    c2b_d = nc.dram_tensor("c2bias", [128, 4, 49], F32, kind="ExternalInput")
    gam_d = nc.dram_tensor("gam", [128, 2], F32, kind="ExternalInput")
    bet_d = nc.dram_tensor("bet", [128, 2], F32, kind="ExternalInput")
    i128_d = nc.dram_tensor("i128", [128, 128], BF16, kind="ExternalInput")
    gm_d = nc.dram_tensor("gmat", [128, 64], F32, kind="ExternalInput")
    em_d = nc.dram_tensor("emat", [64, 128], F32, kind="ExternalInput")
    out_d = nc.dram_tensor("out", [512, S], F32, kind="ExternalOutput")

    AL = mybir.AluOpType
    ACTF = mybir.ActivationFunctionType

    with tile.TileContext(nc) as tc, ExitStack() as ctx:
        if loop_n > 1:
            ctx.enter_context(tc.For_i(0, loop_n, 1))
        consts = ctx.enter_context(tc.tile_pool(name="consts", bufs=1))
        sb = ctx.enter_context(tc.tile_pool(name="sb", bufs=1))
        small = ctx.enter_context(tc.tile_pool(name="small", bufs=8))
        pp = ctx.enter_context(tc.tile_pool(name="pp", bufs=PP_BUFS))
        outp = ctx.enter_context(tc.tile_pool(name="outp", bufs=2))
        phase1_psum = tc.tile_pool(name="psA", bufs=2, space="PSUM")
        psA = phase1_psum.__enter__()
        stat_psum = tc.tile_pool(name="psStat", bufs=1, space="PSUM")
        psStat = stat_psum.__enter__()

        # warm the ACT function tables first: every activation used below
        # (Ln, Exp, Relu, Identity, Copy) lives in one table set, so loading
        # it up-front means zero reloads on the critical path
        warm = consts.tile([1, 1], F32)
        nc.vector.memset(warm, 1.0)
        nc.scalar.activation(out=warm, in_=warm, func=ACTF.Relu)

        # ---- load weights/constants into SBUF
        x_sb = sb.tile([128, 4, S], BF16)
        w1t_sb = sb.tile([128, 4, 256], BF16)
        wvt_sb = sb.tile([128, 4, 512], BF16)
        for kt in range(4):
            nc.sync.dma_start(
                out=x_sb[:, kt, 0:512], in_=x_d[kt * 128 : (kt + 1) * 128, 0:512]
            )
            nc.scalar.dma_start(
                out=w1t_sb[:, kt, :], in_=w1t_d[kt * 128 : (kt + 1) * 128, :]
            )
        for kt in range(4):
            nc.sync.dma_start(
                out=x_sb[:, kt, 512:S], in_=x_d[kt * 128 : (kt + 1) * 128, 512:S]
            )
        for kt in range(4):
            nc.scalar.dma_start(
                out=wvt_sb[:, kt, :], in_=wvt_d[kt * 128 : (kt + 1) * 128, :]
            )
        # small consts go BEFORE the big c2rep stream on the SP ring so the
        # GN-stats matmuls (gmat/emat) aren't stuck behind megabyte copies
        gm_sb = consts.tile([128, 64], F32)
        nc.sync.dma_start(out=gm_sb, in_=gm_d[:])
        em_sb = consts.tile([64, 128], F32)
        nc.sync.dma_start(out=em_sb, in_=em_d[:])
        gam_sb = consts.tile([128, 2], F32)
        nc.sync.dma_start(out=gam_sb, in_=gam_d[:])
        bet_sb = consts.tile([128, 2], F32)
        nc.sync.dma_start(out=bet_sb, in_=bet_d[:])
        c2b_sb = consts.tile([128, 4, 49], F32)
        nc.sync.dma_start(out=c2b_sb, in_=c2b_d[:])
        i128_sb = consts.tile([128, 128], BF16)
        nc.sync.dma_start(out=i128_sb, in_=i128_d[:])
        c2r_sb = sb.tile([128, 2, 49, 128], BF16)
        for sl in range(2):
            for pc in range(4):
                ps0, ps1 = pc * 13, min((pc + 1) * 13, 49)
                eng = nc.sync if sl == 0 else nc.scalar
                eng.dma_start(
                    out=c2r_sb[:, sl, ps0:ps1, :], in_=c2r_d[:, sl, ps0:ps1, :]
                )

        eps_t = consts.tile([64, 1], F32)
        nc.vector.memset(eps_t, EPS)
        magic_t = consts.tile([64, 1], mybir.dt.uint32)
        nc.vector.memset(magic_t, 0x5F3759DF)

        # padded v planes: [r, 38 rows x 40 cols], even parity (A) and the
        # x+1-shifted odd parity (B); zeroed wholesale by Pool under the
        # initial DMA shadow
        vpadA = sb.tile([128, 4, VPLANE], BF16)
        vpadB = sb.tile([128, 4, VPLANE], BF16)
        nc.gpsimd.memset(vpadA.bitcast(mybir.dt.uint16), 0)
        nc.gpsimd.memset(vpadB.bitcast(mybir.dt.uint16), 0)

        # ---- GEMM1: t [256, 1024]; both M-tiles stay in PSUM through GN
        ps_t = []
        for mt in range(2):
            pt = psA.tile([128, S], F32, tag="mm_out")
            ps_t.append(pt)
        for nh in range(2):
            for mt in range(2):
                for kt in range(4):
                    nc.tensor.matmul(
                        ps_t[mt][:, nh * 512 : (nh + 1) * 512],
                        lhsT=w1t_sb[:, kt, mt * 128 : (mt + 1) * 128],
                        rhs=x_sb[:, kt, nh * 512 : (nh + 1) * 512],
                        start=(kt == 0),
                        stop=(kt == 3),
                    )

        # ---- GEMMv: v per rotation -> vpadA interior (bf16); vpadB by DMA
        for r in range(4):
            ps_v = psA.tile([128, S], F32, tag="mm_out")
            for nh in range(2):
                for kt in range(4):
                    nc.tensor.matmul(
                        ps_v[:, nh * 512 : (nh + 1) * 512],
                        lhsT=wvt_sb[:, kt, r * 128 : (r + 1) * 128],
                        rhs=x_sb[:, kt, nh * 512 : (nh + 1) * 512],
                        start=(kt == 0),
                        stop=(kt == 3),
                    )
            vA_int = vpadA[:, r, :].rearrange("q (yy xx) -> q yy xx", xx=VROW)[
                :, 3:35, 3:35
            ]
            nc.scalar.activation(
                out=vA_int,
                in_=ps_v.rearrange("q (y x) -> q y x", x=32),
                func=ACTF.Copy,
            )
            # parity-B: B[y, x] = A[y, x+1] over the region taps can read
            vA_pl = vpadA[:, r, :].rearrange("q (yy xx) -> q yy xx", xx=VROW)
            vB_pl = vpadB[:, r, :].rearrange("q (yy xx) -> q yy xx", xx=VROW)
            nc.sync.dma_start(out=vB_pl[:, :, 0:36], in_=vA_pl[:, :, 1:37])

        # ---- GroupNorm stats (read PSUM directly; m2 assembled in one STT)
        stats = []
        for t in range(2):
            st6 = small.tile([128, 2, 6], F32, tag="st6")
            for hh in range(2):
                nc.vector.bn_stats(
                    out=st6[:, hh, :], in_=ps_t[t][:, hh * 512 : (hh + 1) * 512]
                )
            mv = small.tile([128, 2], F32, tag="mv")
            nc.vector.bn_aggr(out=mv, in_=st6)
            # mv[:,1] <- mean^2 + var  (in-place; mv becomes [mean, m2])
            nc.vector.scalar_tensor_tensor(
                out=mv[:, 1:2],
                in0=mv[:, 0:1],
                scalar=mv[:, 0:1],
                in1=mv[:, 1:2],
                op0=AL.mult,
                op1=AL.add,
            )
            stats.append(mv)

        ps_g = psStat.tile([64, 2], F32, tag="gstat")
        for t in range(2):
            nc.tensor.matmul(
                ps_g, lhsT=gm_sb, rhs=stats[t], start=(t == 0), stop=(t == 1)
            )
        # group mean / m2 -> rstd
        gss = small.tile([64, 2], F32, tag="gss")
        nc.vector.tensor_copy(out=gss, in_=ps_g)  # evacuate PSUM
        gmv = small.tile([64, 2], F32, tag="gmv")  # [mean_g, rstd_g]
        nc.vector.tensor_copy(out=gmv[:, 0:1], in_=gss[:, 0:1])
        gv = small.tile([64, 1], F32, tag="gv")
        nc.vector.tensor_mul(out=gv, in0=gss[:, 0:1], in1=gss[:, 0:1])
        nc.vector.tensor_sub(out=gv, in0=gss[:, 1:2], in1=gv)
        nc.vector.tensor_scalar_add(out=gv, in0=gv, scalar1=EPS)
        # rstd = rsqrt(var+eps) via quake seed + 2 Newton steps, all tiny
        # DVE ops: keeps ACT tables untouched and avoids the 6.5us DVE
        # reciprocal on the critical stats->apply chain
        yq = small.tile([64, 1], F32, tag="yq")
        nc.vector.tensor_single_scalar(
            out=yq.bitcast(mybir.dt.uint32),
            in_=gv.bitcast(mybir.dt.uint32),
            scalar=1,
            op=AL.logical_shift_right,
        )
        nc.vector.tensor_sub(
            out=yq.bitcast(mybir.dt.uint32),
            in0=magic_t.bitcast(mybir.dt.uint32),
            in1=yq.bitcast(mybir.dt.uint32),
        )
        yy = small.tile([64, 1], F32, tag="yy")
        hh_t = small.tile([64, 1], F32, tag="hh_t")
        for it in range(2):
            nc.vector.tensor_mul(out=yy, in0=yq, in1=yq)
            nc.vector.scalar_tensor_tensor(
                out=hh_t, in0=gv, scalar=-0.5, in1=yy, op0=AL.mult, op1=AL.mult
            )
            dst = yq if it == 0 else gmv[:, 1:2]
            nc.vector.scalar_tensor_tensor(
                out=dst, in0=hh_t, scalar=1.5, in1=yq, op0=AL.add, op1=AL.mult
            )

        ps_e = psStat.tile([128, 2], F32, tag="gstat")
        nc.tensor.matmul(ps_e, lhsT=em_sb, rhs=gmv, start=True, stop=True)

        # per-partition scale/bias; apply GN + ReLU into t1 (bf16)
        t1_sb = sb.tile([128, 2, S], BF16)
        scb = small.tile([128, 2, 2], F32, tag="scb")
        for t in range(2):
            nc.vector.tensor_mul(
                out=scb[:, t, 0:1], in0=ps_e[:, 1:2], in1=gam_sb[:, t : t + 1]
            )
            nc.vector.tensor_mul(out=scb[:, t, 1:2], in0=ps_e[:, 0:1], in1=scb[:, t, 0:1])
            nc.vector.tensor_sub(
                out=scb[:, t, 1:2], in0=bet_sb[:, t : t + 1], in1=scb[:, t, 1:2]
            )
            nc.scalar.activation(
                out=t1_sb[:, t, :],
                in_=ps_t[t][:, :],
                func=ACTF.Relu,
                scale=scb[:, t, 0:1],
                bias=scb[:, t, 1:2],
            )

        # phase-1 PSUM pools close here; the involution reuses their banks
        stat_psum.__exit__(None, None, None)
        phase1_psum.__exit__(None, None, None)
        psW = ctx.enter_context(tc.tile_pool(name="psW", bufs=PSW_BUFS, space="PSUM"))
        psO = ctx.enter_context(tc.tile_pool(name="psO", bufs=PSO_BUFS, space="PSUM"))

        routes = _unit_routes()
        u_idx = 0

        # ---- involution: two r-pair streams; a unit is one tap with BOTH
        # spatial halves (nh) fused: the two wrep GEMMs share one LDWEIGHTS,
        # the ACT eviction runs at FD=1024, and the product is one 4D-AP op.
        # Taps alternate r within the pair so the K=64 wrep GEMMs alternate
        # PE row-groups (SBUF partitions 0-63 / 64-127) and can row-tile
        # concurrently on HW.
        for rp, (ra, rb) in enumerate(((0, 1), (2, 3))):
            slot = rp
            out_ps = {}
            units = [(r, p) for r in (ra, rb) for p in range(49)]
            routes_here = routes[u_idx : u_idx + len(units)]
            u_idx += len(units)

            for ui, ((r, p), route) in enumerate(zip(units, routes_here)):
                if p == 0:
                    ops_tile = psO.tile(
                        [128, S], F32, tag="out_ps", name=f"out_ps{r}"
                    )
                    out_ps[r] = ops_tile
                kb = 64 * (r % 2)
                pi, pj = p // 7, p % 7
                par = pj % 2
                vsrc = vpadA if par == 0 else vpadB
                vj = pj - par
                vp = vsrc[:, r, :]
                v_ap = bass.AP(
                    tensor=vp.tensor,
                    offset=vp.offset + pi * VROW + vj,
                    ap=[list(vp.ap[0]), [16 * VROW, 2], [VROW, 16], [1, 32]],
                )
                w_ps = psW.tile([128, S], F32, tag="wrep")
                for nh in (0, 1):
                    nc.tensor.matmul(
                        w_ps[:, nh * 512 : (nh + 1) * 512],
                        lhsT=c2r_sb[kb : kb + 64, slot, p, :],
                        rhs=t1_sb[kb : kb + 64, slot, nh * 512 : (nh + 1) * 512],
                        start=True,
                        stop=True,
                    )
                is_first = p == 0
                is_last = p == 48
                if route == "B":
                    prod = pp.tile([128, S], BF16, tag="prod")
                    nc.vector.scalar_tensor_tensor(
                        out=prod.rearrange("q (t y x) -> q t y x", t=2, x=32),
                        in0=w_ps.rearrange("q (t y x) -> q t y x", t=2, x=32),
                        scalar=c2b_sb[:, r, p : p + 1],
                        in1=v_ap,
                        op0=AL.add,
                        op1=AL.mult,
                    )
                else:  # A (DVE bf16 2x) or C (Pool): ACT evicts wrep + bias
                    w_sb = pp.tile([128, S], BF16, tag="wsb")
                    nc.scalar.activation(
                        out=w_sb,
                        in_=w_ps,
                        func=ACTF.Identity,
                        bias=c2b_sb[:, r, p : p + 1],
                        scale=1.0,
                    )
                    prod = pp.tile([128, S], BF16, tag="prod")
                    if route == "A":
                        nc.vector.tensor_mul(
                            out=prod.rearrange("q (t y x) -> q t y x", t=2, x=32),
                            in0=w_sb.rearrange("q (t y x) -> q t y x", t=2, x=32),
                            in1=v_ap,
                        )
                    else:
                        nc.gpsimd.tensor_mul(
                            out=prod.rearrange("q (t y x) -> q t y x", t=2, x=32),
                            in0=w_sb.rearrange("q (t y x) -> q t y x", t=2, x=32),
                            in1=v_ap,
                        )
                for nh in (0, 1):
                    nc.tensor.matmul(
                        out_ps[r][:, nh * 512 : (nh + 1) * 512],
                        lhsT=i128_sb,
                        rhs=prod[:, nh * 512 : (nh + 1) * 512],
                        start=is_first,
                        stop=is_last,
                    )

            # evacuate PSUM (DMA cannot read PSUM), then scatter to DRAM:
            # out channel (g*16+c, r) -> dram row (g*16+c)*4 + r
            for ri, r in enumerate((ra, rb)):
                out_sb = outp.tile([128, S], F32, tag="out_sb")
                if ri == 0:
                    nc.scalar.copy(out=out_sb, in_=out_ps[r])
                else:
                    nc.vector.tensor_copy(out=out_sb, in_=out_ps[r])
                out_view = out_d[:].rearrange("(o r) s -> r o s", r=4)[r]
                nc.sync.dma_start(out=out_view, in_=out_sb)
            del out_ps

    nc.compile()
    return nc


_CACHED = {}


def _get_module(loop_n=1, fuse=True):
    key = f"nc{loop_n}"
    if key not in _CACHED:
        _CACHED[key] = _build_module(loop_n)
    return _CACHED[key]


# ------------------------------------------------------------------ entrypoint
def _shared_map(prep):
    (W1T, WvT, gam_r, bet_r, c2rep, bias_rep, i128, gmat, emat) = prep
    import ml_dtypes

    W1T = W1T.astype(ml_dtypes.bfloat16)
    WvT = WvT.astype(ml_dtypes.bfloat16)
    c2rep = c2rep.astype(ml_dtypes.bfloat16)
    i128_b = i128.astype(ml_dtypes.bfloat16)
    return {
        "w1t": W1T,
        "wvt": WvT,
        "c2rep": c2rep,
        "c2bias": bias_rep,
        "gam": gam_r,
        "bet": bet_r,
        "i128": i128_b,
        "gmat": gmat,
        "emat": emat,
    }


def _bf16():
    import ml_dtypes

    return ml_dtypes.bfloat16


def kernel(x, v_w, c1_w, gn_g, gn_b, c2_w, c2_b):
    x = np.ascontiguousarray(np.asarray(x, np.float32))
    prep = _host_prep(v_w, c1_w, gn_g, gn_b, c2_w, c2_b)
    nc = _get_module()
    shared = _shared_map(prep)
    in_maps = []
    for c in range(NCORES):
        m = dict(shared)
        m["x"] = np.ascontiguousarray(x[c].reshape(512, S)).astype(_bf16())
        in_maps.append(m)

    res = run_bass_kernel_spmd(nc, in_maps, core_ids=list(range(NCORES)))
    _CACHED["last_results"] = res
    out = np.stack([res.results[c]["out"] for c in range(NCORES)])
    return out.reshape(B, 512, H, W)
